# revision 27
# baseline (speedup 1.0000x reference)
"""Trainium2 Bass kernel for BilinearCategoricalNet.

  h1 = relu(relu(x1 @ m1_w1.T + m1_b1) @ m1_w2.T + m1_b2)      [B, H]
  h2 = same for x2 / m2
  o1 = einsum('bh,oph->bpo', h1, mll1_w) + mll1_b.T            [B, P, O]
  o2 = same for h2 / mll2
  logits = sum_p o1 * o2                                       [B, O]

Strategy: data-parallel over 8 cores (1024 rows each), weights replicated.
All matmuls in fp16 (measured: fp16 512-col matmuls run at ~218 ns vs
~245 ns for fp32r — the fp32r LDWEIGHTS doesn't fully hide in the PE
shadow), f32 PSUM accumulation. Tolerance is 2e-2; fp16-everywhere
lands 7.5e-4.

Activations kept feature-major [h, b] so every contraction has its
contraction dim on SBUF partitions. The MLL stage emits batch-major
[b, (o,p)] PSUM tiles so sum_p becomes a DVE free-axis segmented reduce.
MLL chunk pairs share stationary f-tiles on consecutive matmuls
(measured ~7 ns/instr faster). MLL biases are folded into precomputed
corrections:
  logits = sum_p a*c + h1 @ v1.T + h2 @ v2.T + c0
where a/c are the bias-free MLL outputs, v1[o] = sum_p mll2_b[o,p]*mll1_w[o,p],
v2[o] = sum_p mll1_b[o,p]*mll2_w[o,p], c0[o] = mll1_b[o] . mll2_b[o].
Corrections are computed batch-major directly (stationary = f tile,
moving = v), so no PE transposes are needed.
"""
import sys

sys.path.insert(0, "/opt/trn_rl_repo")

import numpy as np
import ml_dtypes

B = 8192
NCORES = 8
BL = B // NCORES          # 1024 rows per core
NI = 512                  # input features
H = 1024                  # hidden
O = 128                   # num outputs
P = 64                    # pre-bilinear
OP = O * P                # 8192 flattened (o, p), o-major
KC1 = NI // 128           # 4 k-chunks, layer 1
HC = H // 128             # 8 h-chunks
BCH = BL // 512           # 2 batch chunks of 512 (MLP free dim)
BT = BL // 128            # 8 batch tiles of 128 (MLL stationary dim)
CH = OP // 512            # 16 (o,p)-chunks of 512 (= 8 o's each)
CC = CH // 2              # 8 chunk pairs

BF = np.float16
_CACHED = None


def _build():
    import concourse.bacc as bacc
    import concourse.mybir as mybir
    from concourse.tile import TileContext

    f32 = mybir.dt.float32
    fp16 = mybir.dt.float16
    Relu = mybir.ActivationFunctionType.Relu
    Add = mybir.AluOpType.add
    Mult = mybir.AluOpType.mult
    AX = mybir.AxisListType.X

    nc = bacc.Bacc("TRN2", target_bir_lowering=False, debug=False,
                   num_devices=NCORES)

    def din(name, shape, dt=fp16):
        return nc.dram_tensor(name, shape, dt, kind="ExternalInput").ap()

    xT = [din("xT1", [NI, BL]), din("xT2", [NI, BL])]
    w1T = [din("w1T_1", [NI, H]), din("w1T_2", [NI, H])]
    w2T = [din("w2T_1", [H, H]), din("w2T_2", [H, H])]
    b1 = [din("b1_1", [128, HC], f32), din("b1_2", [128, HC], f32)]
    b2 = [din("b2_1", [128, HC], f32), din("b2_2", [128, HC], f32)]
    mllT = [din("mllT1", [H, OP]), din("mllT2", [H, OP])]
    vT = [din("v1T", [H, O]), din("v2T", [H, O])]
    c0 = din("c0", [1, O])
    ones = din("ones", [1, 128])
    out = nc.dram_tensor("out", [BL, O], f32, kind="ExternalOutput").ap()

    with TileContext(nc) as tc:
        with tc.tile_pool(name="persist", bufs=1) as pp:
            # long-lived small tensors (issued first; tiny)
            b1_sb = [pp.tile([128, HC], f32, name=f"b1sb{n}") for n in range(2)]
            b2_sb = [pp.tile([128, HC], f32, name=f"b2sb{n}") for n in range(2)]
            v_sb = [pp.tile([128, HC, O], fp16, name=f"vsb{n}")
                    for n in range(2)]
            c0_sb = pp.tile([1, O], fp16, name="c0sb")
            ones_sb = pp.tile([1, 128], fp16, name="onessb")
            # final MLP outputs, feature-major [h, b] — live through MLL
            f_sb = [pp.tile([128, HC, BL], fp16, name=f"f{n}") for n in range(2)]
            logits_sb = pp.tile([128, BT, O], f32, name="logits_sb")
            corr_sb = pp.tile([128, BT, O], f32, name="corr_sb")

            # ---------------- MLP phase (per net, shared slots) -------------
            with tc.tile_pool(name="mlp", bufs=1) as mp, \
                 tc.tile_pool(name="ps1", bufs=1, space="PSUM") as ps1:
                # PE p-state warmup: the clock needs ~3us of GAPLESS matmul
                # work to reach 2.4 GHz and any sub-us bubble resets it to
                # ~1.2 GHz, so run dummy matmuls until the first layer's
                # inputs have definitely landed (~16us)
                warm = mp.tile([128, 512], fp16, name="warm")
                nc.gpsimd.memset(warm, 0)
                for _ in range(23):
                    pw = ps1.tile([128, 512], f32, name="pw", tag="mlp",
                                  bufs=4)
                    nc.tensor.matmul(pw, warm[:, 0:128], warm, start=True,
                                     stop=True)
                for n in range(2):
                    # spare bufs so net 2's weight DMAs prefetch while net 1's
                    # matmuls still read the old slots
                    x_t = mp.tile([128, KC1, BL], fp16, name=f"x{n}", tag="x_t",
                                  bufs=2)
                    w1_t = mp.tile([128, KC1, H], fp16, name=f"w1{n}",
                                   tag="w1_t", bufs=2)
                    w2_t = mp.tile([128, HC, H], fp16, name=f"w2{n}",
                                   tag="w2_t", bufs=2)
                    xr = xT[n].rearrange("(kc p) b -> p kc b", p=128)
                    w1r = w1T[n].rearrange("(kc p) h -> p kc h", p=128)
                    w2r = w2T[n].rearrange("(kc p) h -> p kc h", p=128)
                    # first-use-ordered loads sized to get several DMA
                    # queues running early (one queue moves ~40-50 GB/s and
                    # each dma_start costs ~0.6us of issue time on Sync —
                    # issuing from the Scalar DGE ring is NOT worth it);
                    # net 0 layer 1 runs bc-outer so its first 8 PSUM groups
                    # only need the bc=0 x halves
                    if n == 0:
                        nc.sync.dma_start(out=w1_t[:, :, 0:256],
                                          in_=w1r[:, :, 0:256])
                        nc.sync.dma_start(out=x_t[:, 0:2, 0:512],
                                          in_=xr[:, 0:2, 0:512])
                        nc.sync.dma_start(out=x_t[:, 2:KC1, 0:512],
                                          in_=xr[:, 2:KC1, 0:512])
                        for q in range(1, 4):
                            nc.sync.dma_start(
                                out=w1_t[:, :, q * 256:(q + 1) * 256],
                                in_=w1r[:, :, q * 256:(q + 1) * 256])
                        nc.sync.dma_start(out=x_t[:, 0:2, 512:BL],
                                          in_=xr[:, 0:2, 512:BL])
                        nc.sync.dma_start(out=x_t[:, 2:KC1, 512:BL],
                                          in_=xr[:, 2:KC1, 512:BL])
                        nc.sync.dma_start(out=b1_sb[n], in_=b1[n])
                        for q in range(4):
                            nc.sync.dma_start(
                                out=w2_t[:, 2 * q:2 * q + 2, :],
                                in_=w2r[:, 2 * q:2 * q + 2, :])
                        nc.sync.dma_start(out=b2_sb[n], in_=b2[n])
                    else:
                        nc.sync.dma_start(out=b1_sb[n], in_=b1[n])
                        nc.sync.dma_start(out=w1_t[:, :, 0:512],
                                          in_=w1r[:, :, 0:512])
                        nc.sync.dma_start(out=w1_t[:, :, 512:H],
                                          in_=w1r[:, :, 512:H])
                        nc.sync.dma_start(out=x_t[:, :, 0:512],
                                          in_=xr[:, :, 0:512])
                        nc.sync.dma_start(out=x_t[:, :, 512:BL],
                                          in_=xr[:, :, 512:BL])
                        nc.sync.dma_start(out=b2_sb[n], in_=b2[n])
                        nc.sync.dma_start(out=w2_t[:, 0:4, :],
                                          in_=w2r[:, 0:4, :])
                        nc.sync.dma_start(out=w2_t[:, 4:HC, :],
                                          in_=w2r[:, 4:HC, :])
                    if n == 1:
                        for q in range(2):
                            nc.sync.dma_start(
                                out=v_sb[q],
                                in_=vT[q].rearrange("(hc p) o -> p hc o",
                                                    p=128))
                        nc.sync.dma_start(out=c0_sb, in_=c0)
                        nc.sync.dma_start(out=ones_sb, in_=ones)
                    h_t = mp.tile([128, HC, BL], fp16, name=f"h{n}",
                                  tag="h_t", bufs=1)
                    # layer 1: h[m] = relu(w1.T @ x + b1). Net 0: bc-outer
                    # (DMA-latency friendly); net 1: bc-pairs share the
                    # stationary w1 column block (data long since resident)
                    if n == 0:
                        for bc in range(BCH):
                            for m in range(HC):
                                pm = ps1.tile([128, 512], f32, name="pm",
                                              tag="mlp", bufs=4)
                                for kc in range(KC1):
                                    nc.tensor.matmul(
                                        pm,
                                        w1_t[:, kc, m * 128:(m + 1) * 128],
                                        x_t[:, kc, bc * 512:(bc + 1) * 512],
                                        start=(kc == 0), stop=(kc == KC1 - 1))
                                nc.scalar.activation(
                                    h_t[:, m, bc * 512:(bc + 1) * 512], pm,
                                    Relu, bias=b1_sb[n][:, m:m + 1])
                    else:
                        for m in range(HC):
                            pm = [ps1.tile([128, 512], f32, name="pm",
                                           tag="mlp", bufs=4)
                                  for _ in range(BCH)]
                            for kc in range(KC1):
                                for bc in range(BCH):
                                    nc.tensor.matmul(
                                        pm[bc],
                                        w1_t[:, kc, m * 128:(m + 1) * 128],
                                        x_t[:, kc, bc * 512:(bc + 1) * 512],
                                        start=(kc == 0), stop=(kc == KC1 - 1))
                            for bc in range(BCH):
                                nc.scalar.activation(
                                    h_t[:, m, bc * 512:(bc + 1) * 512], pm[bc],
                                    Relu, bias=b1_sb[n][:, m:m + 1])
                    # layer 2: f[m] = relu(w2.T @ h + b2)
                    for m in range(HC):
                        pm = [ps1.tile([128, 512], f32, name="pm", tag="mlp",
                                       bufs=4) for _ in range(BCH)]
                        for kc in range(HC):
                            for bc in range(BCH):
                                nc.tensor.matmul(
                                    pm[bc],
                                    w2_t[:, kc, m * 128:(m + 1) * 128],
                                    h_t[:, kc, bc * 512:(bc + 1) * 512],
                                    start=(kc == 0), stop=(kc == HC - 1))
                        for bc in range(BCH):
                            nc.scalar.activation(
                                f_sb[n][:, m, bc * 512:(bc + 1) * 512], pm[bc],
                                Relu, bias=b2_sb[n][:, m:m + 1])

                # ---------- corrections: batch-major h1@v1.T + h2@v2.T + c0
                for bt in range(BT):
                    pc = ps1.tile([128, O], f32, name="pc", tag="mlp", bufs=4)
                    nc.tensor.matmul(pc, ones_sb, c0_sb, start=True,
                                     stop=False)
                    for n in range(2):
                        for hc in range(HC):
                            nc.tensor.matmul(
                                pc, f_sb[n][:, hc, bt * 128:(bt + 1) * 128],
                                v_sb[n][:, hc, :],
                                start=False, stop=(n == 1 and hc == HC - 1))
                    nc.vector.tensor_copy(corr_sb[:, bt, :], pc)

            # ---------------- MLL phase: chunk-pair outer, bt inner ---------
            with tc.tile_pool(name="mll", bufs=1) as lp, \
                 tc.tile_pool(name="ps2", bufs=1, space="PSUM") as ps2:
                for cc in range(CC):
                    m_t = [[lp.tile([128, HC, 512], fp16, name=f"m{n}p{par}",
                                    tag=f"m{n}p{par}", bufs=2)
                            for par in range(2)] for n in range(2)]
                    for n in range(2):
                        for par in range(2):
                            c = 2 * cc + par
                            nc.sync.dma_start(
                                out=m_t[n][par],
                                in_=mllT[n].rearrange("(hc p) f -> p hc f",
                                                      p=128)
                                [:, :, c * 512:(c + 1) * 512])
                    for bt in range(BT):
                        bsl = slice(bt * 128, (bt + 1) * 128)
                        # net 2 first so its PSUM->SBUF copies run under
                        # net 1's matmuls
                        # last tile: par-major so chunk 2cc's DVE drain runs
                        # under chunk 2cc+1's matmuls (shorter exit tail);
                        # elsewhere par-inner so consecutive matmuls share
                        # their stationary f tile (~7 ns/instr faster)
                        tail = (cc == CC - 1 and bt == BT - 1)
                        order = ([(hc, par) for par in range(2)
                                  for hc in range(HC)] if tail else
                                 [(hc, par) for hc in range(HC)
                                  for par in range(2)])
                        pr1 = [ps2.tile([128, 512], f32, name=f"pr1p{par}",
                                        tag=f"pr1p{par}", bufs=2)
                               for par in range(2)]
                        for hc, par in order:
                            nc.tensor.matmul(
                                pr1[par], f_sb[1][:, hc, bsl],
                                m_t[1][par][:, hc, :],
                                start=(hc == 0), stop=(hc == HC - 1))
                        o2_sb = [lp.tile([128, 512], f32, name=f"o2p{par}",
                                         tag=f"o2p{par}", bufs=3)
                                 for par in range(2)]
                        for par in range(2):
                            nc.vector.tensor_copy(o2_sb[par], pr1[par])
                        pr0 = [ps2.tile([128, 512], f32, name=f"pr0p{par}",
                                        tag=f"pr0p{par}", bufs=2)
                               for par in range(2)]
                        if not tail:
                            for hc, par in order:
                                nc.tensor.matmul(
                                    pr0[par], f_sb[0][:, hc, bsl],
                                    m_t[0][par][:, hc, :],
                                    start=(hc == 0), stop=(hc == HC - 1))
                            for par in range(2):
                                c = 2 * cc + par
                                prod = lp.tile([128, 512], f32, name="prod",
                                               tag="prod", bufs=4)
                                nc.vector.tensor_tensor(prod, pr0[par],
                                                        o2_sb[par], op=Mult)
                                nc.vector.tensor_reduce(
                                    logits_sb[:, bt, c * 8:(c + 1) * 8],
                                    prod.rearrange("p (o q) -> p o q", q=P),
                                    axis=AX, op=Add)
                            if cc == CC - 1:
                                o_sb = lp.tile([128, O], f32, name="o_sb",
                                               tag="o_sb", bufs=2)
                                nc.vector.tensor_tensor(
                                    o_sb, logits_sb[:, bt, :],
                                    corr_sb[:, bt, :], op=Add)
                                nc.sync.dma_start(
                                    out=out[bt * 128:(bt + 1) * 128, :],
                                    in_=o_sb)
                        else:
                            # very last tile: par0 runs as one 512-col group
                            # (its drain hides under par1's matmuls); par1
                            # splits into 256-col half-groups and the final
                            # add runs in two pieces, so only ~1us of DVE
                            # work trails the last matmul
                            for hc in range(HC):
                                nc.tensor.matmul(
                                    pr0[0], f_sb[0][:, hc, bsl],
                                    m_t[0][0][:, hc, :],
                                    start=(hc == 0), stop=(hc == HC - 1))
                            prod = lp.tile([128, 512], f32, name="prod",
                                           tag="prod", bufs=4)
                            nc.vector.tensor_tensor(prod, pr0[0], o2_sb[0],
                                                    op=Mult)
                            nc.vector.tensor_reduce(
                                logits_sb[:, bt, (2 * cc) * 8:(2 * cc) * 8 + 8],
                                prod.rearrange("p (o q) -> p o q", q=P),
                                axis=AX, op=Add)
                            o_sb = lp.tile([128, O], f32, name="o_sb",
                                           tag="o_sb", bufs=2)
                            c1b = (2 * cc + 1) * 8
                            # the two half-groups go to separate PSUM tiles
                            # (reusing the long-drained pr1 tags) so half-a's
                            # DVE drain can't false-depend on half-b's matmuls
                            ph = [ps2.tile([128, 256], f32, name=f"ph{hf}",
                                           tag=f"pr1p{hf}", bufs=2)
                                  for hf in range(2)]
                            for hf in range(2):
                                hsl = slice(hf * 256, (hf + 1) * 256)
                                for hc in range(HC):
                                    nc.tensor.matmul(
                                        ph[hf], f_sb[0][:, hc, bsl],
                                        m_t[0][1][:, hc, hsl],
                                        start=(hc == 0), stop=(hc == HC - 1))
                                prodh = lp.tile([128, 256], f32,
                                                name=f"prodh{hf}",
                                                tag=f"prodh{hf}", bufs=1)
                                nc.vector.tensor_tensor(
                                    prodh, ph[hf], o2_sb[1][:, hsl],
                                    op=Mult)
                                nc.vector.tensor_reduce(
                                    logits_sb[:, bt,
                                              c1b + 4 * hf:c1b + 4 * hf + 4],
                                    prodh.rearrange("p (o q) -> p o q", q=P),
                                    axis=AX, op=Add)
                                if hf == 0:
                                    # everything but the last 4 o's summed
                                    # while par1's second half still matmuls
                                    nc.vector.tensor_tensor(
                                        o_sb[:, 0:O - 4],
                                        logits_sb[:, bt, 0:O - 4],
                                        corr_sb[:, bt, 0:O - 4], op=Add)
                                else:
                                    nc.vector.tensor_tensor(
                                        o_sb[:, O - 4:O],
                                        logits_sb[:, bt, O - 4:O],
                                        corr_sb[:, bt, O - 4:O], op=Add)
                            nc.sync.dma_start(
                                out=out[bt * 128:(bt + 1) * 128, :], in_=o_sb)

    nc.compile()
    return nc


def _get_nc():
    global _CACHED
    if _CACHED is None:
        _CACHED = _build()
    return _CACHED


def _prep_shared(m1_w1, m1_b1, m1_w2, m1_b2, m2_w1, m2_b1, m2_w2, m2_b2,
                 mll1_w, mll1_b, mll2_w, mll2_b):
    """Host-side weight layouts, shared by all cores."""
    f = np.float32
    d = {}
    d["w1T_1"] = np.ascontiguousarray(m1_w1.T).astype(BF)
    d["w1T_2"] = np.ascontiguousarray(m2_w1.T).astype(BF)
    d["w2T_1"] = np.ascontiguousarray(m1_w2.T).astype(BF)
    d["w2T_2"] = np.ascontiguousarray(m2_w2.T).astype(BF)
    d["b1_1"] = np.ascontiguousarray(m1_b1.reshape(HC, 128).T).astype(f)
    d["b1_2"] = np.ascontiguousarray(m2_b1.reshape(HC, 128).T).astype(f)
    d["b2_1"] = np.ascontiguousarray(m1_b2.reshape(HC, 128).T).astype(f)
    d["b2_2"] = np.ascontiguousarray(m2_b2.reshape(HC, 128).T).astype(f)
    # [O, P, H] -> [H, O*P] with o-major flattened columns
    d["mllT1"] = np.ascontiguousarray(
        mll1_w.transpose(2, 0, 1).reshape(H, OP)).astype(BF)
    d["mllT2"] = np.ascontiguousarray(
        mll2_w.transpose(2, 0, 1).reshape(H, OP)).astype(BF)
    v1 = np.einsum("op,oph->oh", mll2_b.astype(np.float64),
                   mll1_w.astype(np.float64))
    v2 = np.einsum("op,oph->oh", mll1_b.astype(np.float64),
                   mll2_w.astype(np.float64))
    d["v1T"] = np.ascontiguousarray(v1.T).astype(BF)
    d["v2T"] = np.ascontiguousarray(v2.T).astype(BF)
    d["c0"] = (mll1_b.astype(np.float64) *
               mll2_b.astype(np.float64)).sum(axis=1)[None, :].astype(BF)
    d["ones"] = np.ones((1, 128), dtype=BF)
    return d


def make_in_maps(x_1, x_2, m1_w1, m1_b1, m1_w2, m1_b2, m2_w1, m2_b1, m2_w2,
                 m2_b2, mll1_w, mll1_b, mll2_w, mll2_b):
    shared = _prep_shared(np.asarray(m1_w1), np.asarray(m1_b1),
                          np.asarray(m1_w2), np.asarray(m1_b2),
                          np.asarray(m2_w1), np.asarray(m2_b1),
                          np.asarray(m2_w2), np.asarray(m2_b2),
                          np.asarray(mll1_w), np.asarray(mll1_b),
                          np.asarray(mll2_w), np.asarray(mll2_b))
    x_1 = np.asarray(x_1, dtype=np.float32)
    x_2 = np.asarray(x_2, dtype=np.float32)
    in_maps = []
    for c in range(NCORES):
        sl = slice(c * BL, (c + 1) * BL)
        m = dict(shared)
        m["xT1"] = np.ascontiguousarray(x_1[sl].T).astype(BF)
        m["xT2"] = np.ascontiguousarray(x_2[sl].T).astype(BF)
        in_maps.append(m)
    return in_maps


def kernel(x_1, x_2, m1_w1, m1_b1, m1_w2, m1_b2, m2_w1, m2_b1, m2_w2, m2_b2,
           mll1_w, mll1_b, mll2_w, mll2_b):
    from concourse.bass_utils import run_bass_kernel_spmd

    nc = _get_nc()
    in_maps = make_in_maps(x_1, x_2, m1_w1, m1_b1, m1_w2, m1_b2, m2_w1,
                           m2_b1, m2_w2, m2_b2, mll1_w, mll1_b, mll2_w,
                           mll2_b)
    res = run_bass_kernel_spmd(nc, in_maps, list(range(NCORES)))
    return np.concatenate([res.results[c]["out"] for c in range(NCORES)],
                          axis=0)


# revision 28
# speedup vs baseline: 1.0083x; 1.0083x over previous
"""Trainium2 Bass kernel for BilinearCategoricalNet.

  h1 = relu(relu(x1 @ m1_w1.T + m1_b1) @ m1_w2.T + m1_b2)      [B, H]
  h2 = same for x2 / m2
  o1 = einsum('bh,oph->bpo', h1, mll1_w) + mll1_b.T            [B, P, O]
  o2 = same for h2 / mll2
  logits = sum_p o1 * o2                                       [B, O]

Strategy: data-parallel over 8 cores (1024 rows each), weights replicated.
All matmuls in fp16 (measured: fp16 512-col matmuls run at ~218 ns vs
~245 ns for fp32r — the fp32r LDWEIGHTS doesn't fully hide in the PE
shadow), f32 PSUM accumulation. Tolerance is 2e-2; fp16-everywhere
lands 7.5e-4.

Activations kept feature-major [h, b] so every contraction has its
contraction dim on SBUF partitions. The MLL stage emits batch-major
[b, (o,p)] PSUM tiles so sum_p becomes a DVE free-axis segmented reduce.
MLL chunk pairs share stationary f-tiles on consecutive matmuls
(measured ~7 ns/instr faster). MLL biases are folded into precomputed
corrections:
  logits = sum_p a*c + h1 @ v1.T + h2 @ v2.T + c0
where a/c are the bias-free MLL outputs, v1[o] = sum_p mll2_b[o,p]*mll1_w[o,p],
v2[o] = sum_p mll1_b[o,p]*mll2_w[o,p], c0[o] = mll1_b[o] . mll2_b[o].
Corrections are computed batch-major directly (stationary = f tile,
moving = v), so no PE transposes are needed.
"""
import sys

sys.path.insert(0, "/opt/trn_rl_repo")

import numpy as np
import ml_dtypes

B = 8192
NCORES = 8
BL = B // NCORES          # 1024 rows per core
NI = 512                  # input features
H = 1024                  # hidden
O = 128                   # num outputs
P = 64                    # pre-bilinear
OP = O * P                # 8192 flattened (o, p), o-major
KC1 = NI // 128           # 4 k-chunks, layer 1
HC = H // 128             # 8 h-chunks
BCH = BL // 512           # 2 batch chunks of 512 (MLP free dim)
BT = BL // 128            # 8 batch tiles of 128 (MLL stationary dim)
CH = OP // 512            # 16 (o,p)-chunks of 512 (= 8 o's each)
CC = CH // 2              # 8 chunk pairs

BF = np.float16
_CACHED = None


def _build():
    import concourse.bacc as bacc
    import concourse.mybir as mybir
    from concourse.tile import TileContext

    f32 = mybir.dt.float32
    fp16 = mybir.dt.float16
    Relu = mybir.ActivationFunctionType.Relu
    Add = mybir.AluOpType.add
    Mult = mybir.AluOpType.mult
    AX = mybir.AxisListType.X

    nc = bacc.Bacc("TRN2", target_bir_lowering=False, debug=False,
                   num_devices=NCORES)

    def din(name, shape, dt=fp16):
        return nc.dram_tensor(name, shape, dt, kind="ExternalInput").ap()

    xT = [din("xT1", [NI, BL]), din("xT2", [NI, BL])]
    w1T = [din("w1T_1", [NI, H]), din("w1T_2", [NI, H])]
    w2T = [din("w2T_1", [H, H]), din("w2T_2", [H, H])]
    b1 = [din("b1_1", [128, HC], f32), din("b1_2", [128, HC], f32)]
    b2 = [din("b2_1", [128, HC], f32), din("b2_2", [128, HC], f32)]
    mllT = [din("mllT1", [H, OP]), din("mllT2", [H, OP])]
    vT = [din("v1T", [H, O]), din("v2T", [H, O])]
    c0 = din("c0", [1, O])
    ones = din("ones", [1, 128])
    out = nc.dram_tensor("out", [BL, O], f32, kind="ExternalOutput").ap()

    with TileContext(nc) as tc:
        with tc.tile_pool(name="persist", bufs=1) as pp:
            # long-lived small tensors (issued first; tiny)
            b1_sb = [pp.tile([128, HC], f32, name=f"b1sb{n}") for n in range(2)]
            b2_sb = [pp.tile([128, HC], f32, name=f"b2sb{n}") for n in range(2)]
            v_sb = [pp.tile([128, HC, O], fp16, name=f"vsb{n}")
                    for n in range(2)]
            c0_sb = pp.tile([1, O], fp16, name="c0sb")
            ones_sb = pp.tile([1, 128], fp16, name="onessb")
            # final MLP outputs, feature-major [h, b] — live through MLL
            f_sb = [pp.tile([128, HC, BL], fp16, name=f"f{n}") for n in range(2)]
            logits_sb = pp.tile([128, BT, O], f32, name="logits_sb")
            corr_sb = pp.tile([128, BT, O], f32, name="corr_sb")

            # ---------------- MLP phase (per net, shared slots) -------------
            with tc.tile_pool(name="mlp", bufs=1) as mp, \
                 tc.tile_pool(name="ps1", bufs=1, space="PSUM") as ps1:
                # PE p-state warmup: the clock needs ~3us of GAPLESS matmul
                # work to reach 2.4 GHz and any sub-us bubble resets it to
                # ~1.2 GHz, so run dummy matmuls until the first layer's
                # inputs have definitely landed (~16us)
                warm = mp.tile([128, 512], fp16, name="warm")
                nc.gpsimd.memset(warm, 0)
                for _ in range(23):
                    pw = ps1.tile([128, 512], f32, name="pw", tag="mlp",
                                  bufs=4)
                    nc.tensor.matmul(pw, warm[:, 0:128], warm, start=True,
                                     stop=True)
                for n in range(2):
                    # spare bufs so net 2's weight DMAs prefetch while net 1's
                    # matmuls still read the old slots
                    x_t = mp.tile([128, KC1, BL], fp16, name=f"x{n}", tag="x_t",
                                  bufs=2)
                    w1_t = mp.tile([128, KC1, H], fp16, name=f"w1{n}",
                                   tag="w1_t", bufs=2)
                    w2_t = mp.tile([128, HC, H], fp16, name=f"w2{n}",
                                   tag="w2_t", bufs=2)
                    xr = xT[n].rearrange("(kc p) b -> p kc b", p=128)
                    w1r = w1T[n].rearrange("(kc p) h -> p kc h", p=128)
                    w2r = w2T[n].rearrange("(kc p) h -> p kc h", p=128)
                    # first-use-ordered loads sized to get several DMA
                    # queues running early (one queue moves ~40-50 GB/s and
                    # each dma_start costs ~0.6us of issue time on Sync —
                    # issuing from the Scalar DGE ring is NOT worth it);
                    # net 0 layer 1 runs bc-outer so its first 8 PSUM groups
                    # only need the bc=0 x halves
                    if n == 0:
                        nc.sync.dma_start(out=w1_t[:, :, 0:256],
                                          in_=w1r[:, :, 0:256])
                        nc.sync.dma_start(out=x_t[:, 0:2, 0:512],
                                          in_=xr[:, 0:2, 0:512])
                        nc.sync.dma_start(out=x_t[:, 2:KC1, 0:512],
                                          in_=xr[:, 2:KC1, 0:512])
                        for q in range(1, 4):
                            nc.sync.dma_start(
                                out=w1_t[:, :, q * 256:(q + 1) * 256],
                                in_=w1r[:, :, q * 256:(q + 1) * 256])
                        nc.sync.dma_start(out=x_t[:, 0:2, 512:BL],
                                          in_=xr[:, 0:2, 512:BL])
                        nc.sync.dma_start(out=x_t[:, 2:KC1, 512:BL],
                                          in_=xr[:, 2:KC1, 512:BL])
                        nc.sync.dma_start(out=b1_sb[n], in_=b1[n])
                        for q in range(4):
                            nc.sync.dma_start(
                                out=w2_t[:, 2 * q:2 * q + 2, :],
                                in_=w2r[:, 2 * q:2 * q + 2, :])
                        nc.sync.dma_start(out=b2_sb[n], in_=b2[n])
                    else:
                        nc.sync.dma_start(out=b1_sb[n], in_=b1[n])
                        nc.sync.dma_start(out=w1_t[:, :, 0:512],
                                          in_=w1r[:, :, 0:512])
                        nc.sync.dma_start(out=w1_t[:, :, 512:H],
                                          in_=w1r[:, :, 512:H])
                        nc.sync.dma_start(out=x_t[:, :, 0:512],
                                          in_=xr[:, :, 0:512])
                        nc.sync.dma_start(out=x_t[:, :, 512:BL],
                                          in_=xr[:, :, 512:BL])
                        nc.sync.dma_start(out=b2_sb[n], in_=b2[n])
                        nc.sync.dma_start(out=w2_t[:, 0:4, :],
                                          in_=w2r[:, 0:4, :])
                        nc.sync.dma_start(out=w2_t[:, 4:HC, :],
                                          in_=w2r[:, 4:HC, :])
                    if n == 1:
                        for q in range(2):
                            nc.sync.dma_start(
                                out=v_sb[q],
                                in_=vT[q].rearrange("(hc p) o -> p hc o",
                                                    p=128))
                        nc.sync.dma_start(out=c0_sb, in_=c0)
                        nc.sync.dma_start(out=ones_sb, in_=ones)
                    h_t = mp.tile([128, HC, BL], fp16, name=f"h{n}",
                                  tag="h_t", bufs=1)
                    # layer 1: h[m] = relu(w1.T @ x + b1). Net 0: bc-outer
                    # (DMA-latency friendly); net 1: bc-pairs share the
                    # stationary w1 column block (data long since resident)
                    if n == 0:
                        for bc in range(BCH):
                            for m in range(HC):
                                pm = ps1.tile([128, 512], f32, name="pm",
                                              tag="mlp", bufs=4)
                                for kc in range(KC1):
                                    nc.tensor.matmul(
                                        pm,
                                        w1_t[:, kc, m * 128:(m + 1) * 128],
                                        x_t[:, kc, bc * 512:(bc + 1) * 512],
                                        start=(kc == 0), stop=(kc == KC1 - 1))
                                nc.scalar.activation(
                                    h_t[:, m, bc * 512:(bc + 1) * 512], pm,
                                    Relu, bias=b1_sb[n][:, m:m + 1])
                    else:
                        for m in range(HC):
                            pm = [ps1.tile([128, 512], f32, name="pm",
                                           tag="mlp", bufs=4)
                                  for _ in range(BCH)]
                            for kc in range(KC1):
                                for bc in range(BCH):
                                    nc.tensor.matmul(
                                        pm[bc],
                                        w1_t[:, kc, m * 128:(m + 1) * 128],
                                        x_t[:, kc, bc * 512:(bc + 1) * 512],
                                        start=(kc == 0), stop=(kc == KC1 - 1))
                            for bc in range(BCH):
                                nc.scalar.activation(
                                    h_t[:, m, bc * 512:(bc + 1) * 512], pm[bc],
                                    Relu, bias=b1_sb[n][:, m:m + 1])
                    # layer 2: f[m] = relu(w2.T @ h + b2)
                    for m in range(HC):
                        pm = [ps1.tile([128, 512], f32, name="pm", tag="mlp",
                                       bufs=4) for _ in range(BCH)]
                        for kc in range(HC):
                            for bc in range(BCH):
                                nc.tensor.matmul(
                                    pm[bc],
                                    w2_t[:, kc, m * 128:(m + 1) * 128],
                                    h_t[:, kc, bc * 512:(bc + 1) * 512],
                                    start=(kc == 0), stop=(kc == HC - 1))
                        for bc in range(BCH):
                            nc.scalar.activation(
                                f_sb[n][:, m, bc * 512:(bc + 1) * 512], pm[bc],
                                Relu, bias=b2_sb[n][:, m:m + 1])

                # ---------- corrections: batch-major h1@v1.T + h2@v2.T + c0
                for bt in range(BT):
                    pc = ps1.tile([128, O], f32, name="pc", tag="mlp", bufs=4)
                    nc.tensor.matmul(pc, ones_sb, c0_sb, start=True,
                                     stop=False)
                    for n in range(2):
                        for hc in range(HC):
                            nc.tensor.matmul(
                                pc, f_sb[n][:, hc, bt * 128:(bt + 1) * 128],
                                v_sb[n][:, hc, :],
                                start=False, stop=(n == 1 and hc == HC - 1))
                    nc.vector.tensor_copy(corr_sb[:, bt, :], pc)

            # ---------------- MLL phase: chunk-pair outer, bt inner ---------
            with tc.tile_pool(name="mll", bufs=1) as lp, \
                 tc.tile_pool(name="ps2", bufs=1, space="PSUM") as ps2:
                for cc in range(CC):
                    m_t = [[lp.tile([128, HC, 512], fp16, name=f"m{n}p{par}",
                                    tag=f"m{n}p{par}", bufs=2)
                            for par in range(2)] for n in range(2)]
                    # net 1's tiles are consumed first in the bt loop, so
                    # issue their loads first
                    for n in (1, 0):
                        for par in range(2):
                            c = 2 * cc + par
                            nc.sync.dma_start(
                                out=m_t[n][par],
                                in_=mllT[n].rearrange("(hc p) f -> p hc f",
                                                      p=128)
                                [:, :, c * 512:(c + 1) * 512])
                    for bt in range(BT):
                        bsl = slice(bt * 128, (bt + 1) * 128)
                        # net 2 first so its PSUM->SBUF copies run under
                        # net 1's matmuls
                        # last tile: par-major so chunk 2cc's DVE drain runs
                        # under chunk 2cc+1's matmuls (shorter exit tail);
                        # elsewhere par-inner so consecutive matmuls share
                        # their stationary f tile (~7 ns/instr faster)
                        tail = (cc == CC - 1 and bt == BT - 1)
                        order = ([(hc, par) for par in range(2)
                                  for hc in range(HC)] if tail else
                                 [(hc, par) for hc in range(HC)
                                  for par in range(2)])
                        pr1 = [ps2.tile([128, 512], f32, name=f"pr1p{par}",
                                        tag=f"pr1p{par}", bufs=2)
                               for par in range(2)]
                        for hc, par in order:
                            nc.tensor.matmul(
                                pr1[par], f_sb[1][:, hc, bsl],
                                m_t[1][par][:, hc, :],
                                start=(hc == 0), stop=(hc == HC - 1))
                        o2_sb = [lp.tile([128, 512], f32, name=f"o2p{par}",
                                         tag=f"o2p{par}", bufs=3)
                                 for par in range(2)]
                        for par in range(2):
                            nc.vector.tensor_copy(o2_sb[par], pr1[par])
                        pr0 = [ps2.tile([128, 512], f32, name=f"pr0p{par}",
                                        tag=f"pr0p{par}", bufs=2)
                               for par in range(2)]
                        if not tail:
                            for hc, par in order:
                                nc.tensor.matmul(
                                    pr0[par], f_sb[0][:, hc, bsl],
                                    m_t[0][par][:, hc, :],
                                    start=(hc == 0), stop=(hc == HC - 1))
                            for par in range(2):
                                c = 2 * cc + par
                                prod = lp.tile([128, 512], f32, name="prod",
                                               tag="prod", bufs=4)
                                nc.vector.tensor_tensor(prod, pr0[par],
                                                        o2_sb[par], op=Mult)
                                nc.vector.tensor_reduce(
                                    logits_sb[:, bt, c * 8:(c + 1) * 8],
                                    prod.rearrange("p (o q) -> p o q", q=P),
                                    axis=AX, op=Add)
                            if cc == CC - 1:
                                o_sb = lp.tile([128, O], f32, name="o_sb",
                                               tag="o_sb", bufs=2)
                                nc.vector.tensor_tensor(
                                    o_sb, logits_sb[:, bt, :],
                                    corr_sb[:, bt, :], op=Add)
                                nc.sync.dma_start(
                                    out=out[bt * 128:(bt + 1) * 128, :],
                                    in_=o_sb)
                        else:
                            # very last tile: par0 runs as one 512-col group
                            # (its drain hides under par1's matmuls); par1
                            # splits into 256-col half-groups and the final
                            # add runs in two pieces, so only ~1us of DVE
                            # work trails the last matmul
                            for hc in range(HC):
                                nc.tensor.matmul(
                                    pr0[0], f_sb[0][:, hc, bsl],
                                    m_t[0][0][:, hc, :],
                                    start=(hc == 0), stop=(hc == HC - 1))
                            prod = lp.tile([128, 512], f32, name="prod",
                                           tag="prod", bufs=4)
                            nc.vector.tensor_tensor(prod, pr0[0], o2_sb[0],
                                                    op=Mult)
                            nc.vector.tensor_reduce(
                                logits_sb[:, bt, (2 * cc) * 8:(2 * cc) * 8 + 8],
                                prod.rearrange("p (o q) -> p o q", q=P),
                                axis=AX, op=Add)
                            o_sb = lp.tile([128, O], f32, name="o_sb",
                                           tag="o_sb", bufs=2)
                            c1b = (2 * cc + 1) * 8
                            # the two half-groups go to separate PSUM tiles
                            # (reusing the long-drained pr1 tags) so half-a's
                            # DVE drain can't false-depend on half-b's matmuls
                            ph = [ps2.tile([128, 256], f32, name=f"ph{hf}",
                                           tag=f"pr1p{hf}", bufs=2)
                                  for hf in range(2)]
                            for hf in range(2):
                                hsl = slice(hf * 256, (hf + 1) * 256)
                                for hc in range(HC):
                                    nc.tensor.matmul(
                                        ph[hf], f_sb[0][:, hc, bsl],
                                        m_t[0][1][:, hc, hsl],
                                        start=(hc == 0), stop=(hc == HC - 1))
                                prodh = lp.tile([128, 256], f32,
                                                name=f"prodh{hf}",
                                                tag=f"prodh{hf}", bufs=1)
                                nc.vector.tensor_tensor(
                                    prodh, ph[hf], o2_sb[1][:, hsl],
                                    op=Mult)
                                nc.vector.tensor_reduce(
                                    logits_sb[:, bt,
                                              c1b + 4 * hf:c1b + 4 * hf + 4],
                                    prodh.rearrange("p (o q) -> p o q", q=P),
                                    axis=AX, op=Add)
                                if hf == 0:
                                    # everything but the last 4 o's summed
                                    # while par1's second half still matmuls
                                    nc.vector.tensor_tensor(
                                        o_sb[:, 0:O - 4],
                                        logits_sb[:, bt, 0:O - 4],
                                        corr_sb[:, bt, 0:O - 4], op=Add)
                                else:
                                    nc.vector.tensor_tensor(
                                        o_sb[:, O - 4:O],
                                        logits_sb[:, bt, O - 4:O],
                                        corr_sb[:, bt, O - 4:O], op=Add)
                            nc.sync.dma_start(
                                out=out[bt * 128:(bt + 1) * 128, :], in_=o_sb)

    nc.compile()
    return nc


def _get_nc():
    global _CACHED
    if _CACHED is None:
        _CACHED = _build()
    return _CACHED


def _prep_shared(m1_w1, m1_b1, m1_w2, m1_b2, m2_w1, m2_b1, m2_w2, m2_b2,
                 mll1_w, mll1_b, mll2_w, mll2_b):
    """Host-side weight layouts, shared by all cores."""
    f = np.float32
    d = {}
    d["w1T_1"] = np.ascontiguousarray(m1_w1.T).astype(BF)
    d["w1T_2"] = np.ascontiguousarray(m2_w1.T).astype(BF)
    d["w2T_1"] = np.ascontiguousarray(m1_w2.T).astype(BF)
    d["w2T_2"] = np.ascontiguousarray(m2_w2.T).astype(BF)
    d["b1_1"] = np.ascontiguousarray(m1_b1.reshape(HC, 128).T).astype(f)
    d["b1_2"] = np.ascontiguousarray(m2_b1.reshape(HC, 128).T).astype(f)
    d["b2_1"] = np.ascontiguousarray(m1_b2.reshape(HC, 128).T).astype(f)
    d["b2_2"] = np.ascontiguousarray(m2_b2.reshape(HC, 128).T).astype(f)
    # [O, P, H] -> [H, O*P] with o-major flattened columns
    d["mllT1"] = np.ascontiguousarray(
        mll1_w.transpose(2, 0, 1).reshape(H, OP)).astype(BF)
    d["mllT2"] = np.ascontiguousarray(
        mll2_w.transpose(2, 0, 1).reshape(H, OP)).astype(BF)
    v1 = np.einsum("op,oph->oh", mll2_b.astype(np.float64),
                   mll1_w.astype(np.float64))
    v2 = np.einsum("op,oph->oh", mll1_b.astype(np.float64),
                   mll2_w.astype(np.float64))
    d["v1T"] = np.ascontiguousarray(v1.T).astype(BF)
    d["v2T"] = np.ascontiguousarray(v2.T).astype(BF)
    d["c0"] = (mll1_b.astype(np.float64) *
               mll2_b.astype(np.float64)).sum(axis=1)[None, :].astype(BF)
    d["ones"] = np.ones((1, 128), dtype=BF)
    return d


def make_in_maps(x_1, x_2, m1_w1, m1_b1, m1_w2, m1_b2, m2_w1, m2_b1, m2_w2,
                 m2_b2, mll1_w, mll1_b, mll2_w, mll2_b):
    shared = _prep_shared(np.asarray(m1_w1), np.asarray(m1_b1),
                          np.asarray(m1_w2), np.asarray(m1_b2),
                          np.asarray(m2_w1), np.asarray(m2_b1),
                          np.asarray(m2_w2), np.asarray(m2_b2),
                          np.asarray(mll1_w), np.asarray(mll1_b),
                          np.asarray(mll2_w), np.asarray(mll2_b))
    x_1 = np.asarray(x_1, dtype=np.float32)
    x_2 = np.asarray(x_2, dtype=np.float32)
    in_maps = []
    for c in range(NCORES):
        sl = slice(c * BL, (c + 1) * BL)
        m = dict(shared)
        m["xT1"] = np.ascontiguousarray(x_1[sl].T).astype(BF)
        m["xT2"] = np.ascontiguousarray(x_2[sl].T).astype(BF)
        in_maps.append(m)
    return in_maps


def kernel(x_1, x_2, m1_w1, m1_b1, m1_w2, m1_b2, m2_w1, m2_b1, m2_w2, m2_b2,
           mll1_w, mll1_b, mll2_w, mll2_b):
    from concourse.bass_utils import run_bass_kernel_spmd

    nc = _get_nc()
    in_maps = make_in_maps(x_1, x_2, m1_w1, m1_b1, m1_w2, m1_b2, m2_w1,
                           m2_b1, m2_w2, m2_b2, mll1_w, mll1_b, mll2_w,
                           mll2_b)
    res = run_bass_kernel_spmd(nc, in_maps, list(range(NCORES)))
    return np.concatenate([res.results[c]["out"] for c in range(NCORES)],
                          axis=0)
